# revision 7
# baseline (speedup 1.0000x reference)
"""Trainium2 Bass kernel for nn_L3_31799937859925 (sparse_attention).

Strategy:
- Each query row (label = seq_sort[j] in [0,64)) attends only to kv rows with
  emb_alloc == label, so we sort queries by label on the host and give each of
  the 8 cores a contiguous 2048-query slice (pure data parallel, no
  collectives). kv rows are label-sorted too, so each 512-query tile only needs
  a small contiguous kv window (W columns) + an additive -1e30 mask bias.
- On device everything is feature-major ([feature, query]) so no transposes are
  needed: scoresT = K'T @ x, softmax sums / rms stats via ones-column matmuls
  on the PE, per-query scalars broadcast across partitions via K=1 matmuls.
- norm_in_weight is folded into w_k, norm_out_weight into w_mix (host side).
- All heavy matmuls run in float32r (relaxed fp32, full PE rate, ~1.5e-4 rel).
"""
import numpy as np

import concourse.bass as bass
import concourse.tile as tile
from concourse import bacc, mybir
import concourse.bass_utils as bass_utils

F32 = mybir.dt.float32
F32R = mybir.dt.float32r
AF = mybir.ActivationFunctionType
MUL = mybir.AluOpType.mult
ADD = mybir.AluOpType.add

H, N_EMB, D_EMB, D_UP = 1024, 8192, 512, 2048
B, T = 4, 4096
BT = B * T                  # 16384
NC = 8                      # cores
NQ = BT // NC               # 2048 queries per core
QT = 512                    # queries per q-tile
NQT = NQ // QT              # 4 q-tiles per core
HC = H // 128               # 8
DC = D_EMB // 128           # 4
JC = D_UP // 128            # 16
KC = (D_UP + H) // 128      # 24 contraction chunks for mix
MC = H // 128               # 8 output chunks

LAST_RESULTS = None         # BassKernelResults of the most recent run (for test.py)
LAST_EXEC_S = None
_PROGRAM_CACHE = {}


def _build_program(W):
    """Build the SPMD single-core program. W = kv window width (mult of 128)."""
    n_kvc = W // 128
    nc = bacc.Bacc("TRN2", target_bir_lowering=False, debug=False,
                   enable_asserts=False)

    x_in = nc.dram_tensor("x_in", [128, HC, NQ], F32R, kind="ExternalInput")
    kt_in = nc.dram_tensor("kt_in", [NQT, 128, HC, W], F32R, kind="ExternalInput")
    v_in = nc.dram_tensor("v_in", [NQT, 128, n_kvc, D_EMB], F32R, kind="ExternalInput")
    b_in = nc.dram_tensor("b_in", [NQT, 128, n_kvc, QT], F32, kind="ExternalInput")
    wup_in = nc.dram_tensor("wup_in", [128, DC, D_UP], F32R, kind="ExternalInput")
    wmix_in = nc.dram_tensor("wmix_in", [MC, 128, KC, 128], F32R, kind="ExternalInput")
    out_d = nc.dram_tensor("out_d", [MC, 128, NQ], F32, kind="ExternalOutput")

    from contextlib import ExitStack
    with tile.TileContext(nc) as tc, ExitStack() as ctx:
        ec = ctx.enter_context
        cst = ec(tc.tile_pool(name="cst", bufs=1))
        pwup = ec(tc.tile_pool(name="wup", bufs=1))
        px = ec(tc.tile_pool(name="px", bufs=2))
        pkt = ec(tc.tile_pool(name="pkt", bufs=1))
        pv = ec(tc.tile_pool(name="pv", bufs=1))
        pb = ec(tc.tile_pool(name="pb", bufs=1))
        pwm = ec(tc.tile_pool(name="pwm", bufs=3))
        px2 = ec(tc.tile_pool(name="px2", bufs=2))
        ppu = ec(tc.tile_pool(name="ppu", bufs=1))
        pt = ec(tc.tile_pool(name="pt", bufs=2))
        pcomb = ec(tc.tile_pool(name="pcomb", bufs=1))
        pup = ec(tc.tile_pool(name="pup", bufs=1))
        pu2 = ec(tc.tile_pool(name="pu2", bufs=3))
        pbc = ec(tc.tile_pool(name="pbc", bufs=3))
        prows = ec(tc.tile_pool(name="prows", bufs=3))
        po = ec(tc.tile_pool(name="po", bufs=2))
        pbig = ec(tc.tile_pool(name="pbig", bufs=6, space="PSUM"))
        prow = ec(tc.tile_pool(name="prow", bufs=2, space="PSUM"))

        if True:
            ones_f = cst.tile([128, 1], F32)
            nc.vector.memset(ones_f, 1.0)
            ones_col = cst.tile([128, 1], F32R)
            nc.vector.tensor_copy(ones_col, ones_f)
            ones_rf = cst.tile([1, 128], F32)
            nc.vector.memset(ones_rf, 1.0)
            ones_row = cst.tile([1, 128], F32R)
            nc.vector.tensor_copy(ones_row, ones_rf)
            eps_t = cst.tile([128, 1], F32)
            nc.vector.memset(eps_t, 1e-6)

            wup_sb = pwup.tile([128, DC, D_UP], F32R)
            nc.sync.dma_start(wup_sb[:], wup_in.ap())

            for qt in range(NQT):
                qs = slice(qt * QT, (qt + 1) * QT)
                x_t = px.tile([128, HC, QT], F32R, tag="x")
                nc.sync.dma_start(x_t[:], x_in.ap()[:, :, qs])
                kt_t = pkt.tile([128, HC, W], F32R, tag="kt")
                nc.sync.dma_start(kt_t[:], kt_in.ap()[qt])
                v_t = pv.tile([128, n_kvc, D_EMB], F32R, tag="v")
                nc.sync.dma_start(v_t[:], v_in.ap()[qt])
                b_t = pb.tile([128, n_kvc, QT], F32, tag="b")
                nc.sync.dma_start(b_t[:], b_in.ap()[qt])

                # ---- rms_in stats: inv_rms per query as broadcast [128, QT]
                ss_ps = prow.tile([1, QT], F32, tag="row")
                for hc in range(HC):
                    x2 = px2.tile([128, QT], F32R, tag="x2")
                    nc.scalar.activation(x2, x_t[:, hc, :].bitcast(F32), AF.Square)
                    nc.tensor.matmul(ss_ps, lhsT=ones_col, rhs=x2,
                                     start=(hc == 0), stop=(hc == HC - 1))
                sd = prows.tile([1, QT], F32, tag="rows")
                nc.scalar.activation(sd, ss_ps, AF.Sqrt, bias=eps_t[:1],
                                     scale=1.0 / H)
                crf = prows.tile([1, QT], F32, tag="rows")
                nc.vector.reciprocal(crf, sd)
                cr = prows.tile([1, QT], F32R, tag="rowsr")
                nc.vector.tensor_copy(cr, crf)
                c_b = pbc.tile([128, QT], F32, tag="bc")

                # ---- scoresT [kv, q] per kv chunk; t = s*c + bias; pu = exp(t)
                pu_t = ppu.tile([128, n_kvc, QT], F32R, tag="pu")
                for kvc in range(n_kvc):
                    s_ps = pbig.tile([128, QT], F32, tag="big")
                    for hc in range(HC):
                        nc.tensor.matmul(
                            s_ps, lhsT=kt_t[:, hc, kvc * 128:(kvc + 1) * 128],
                            rhs=x_t[:, hc, :],
                            start=(hc == 0), stop=(hc == HC - 1))
                    if kvc == 0:
                        # emit bcast here so PE doesn't stall on the recip chain
                        cb_ps = pbig.tile([128, QT], F32, tag="big")
                        nc.tensor.matmul(cb_ps, lhsT=ones_row, rhs=cr,
                                         start=True, stop=True)
                        nc.vector.tensor_copy(c_b, cb_ps)
                    t_sb = pt.tile([128, QT], F32, tag="t")
                    nc.vector.tensor_tensor(t_sb, s_ps, c_b, MUL)
                    nc.vector.tensor_tensor(t_sb, t_sb, b_t[:, kvc, :], ADD)
                    nc.scalar.activation(pu_t[:, kvc, :], t_sb, AF.Exp)

                # ---- z = sum_kv pu ; z_b = 1/z broadcast
                z_ps = prow.tile([1, QT], F32, tag="row")
                for kvc in range(n_kvc):
                    nc.tensor.matmul(z_ps, lhsT=ones_col, rhs=pu_t[:, kvc, :],
                                     start=(kvc == 0), stop=(kvc == n_kvc - 1))
                zrf = prows.tile([1, QT], F32, tag="rows")
                nc.vector.reciprocal(zrf, z_ps)
                zr = prows.tile([1, QT], F32R, tag="rowsr")
                nc.vector.tensor_copy(zr, zrf)
                z_b = pbc.tile([128, QT], F32, tag="bc")

                # ---- combT [d, q] = V^T pu, normalized by z
                comb_t = pcomb.tile([128, DC, QT], F32R, tag="comb")
                c_pss = []
                for dc in range(DC):
                    c_ps = pbig.tile([128, QT], F32, tag="big")
                    for kvc in range(n_kvc):
                        nc.tensor.matmul(
                            c_ps, lhsT=v_t[:, kvc, dc * 128:(dc + 1) * 128],
                            rhs=pu_t[:, kvc, :],
                            start=(kvc == 0), stop=(kvc == n_kvc - 1))
                    if dc == 0:
                        zb_ps = pbig.tile([128, QT], F32, tag="big")
                        nc.tensor.matmul(zb_ps, lhsT=ones_row, rhs=zr,
                                         start=True, stop=True)
                        nc.vector.tensor_copy(z_b, zb_ps)
                    nc.vector.tensor_tensor(comb_t[:, dc, :], c_ps, z_b, MUL)

                # ---- upT [j, q] (raw, pre-norm) + sum of squares
                up_t = pup.tile([128, JC, QT], F32R, tag="up")
                ssu_ps = prow.tile([1, QT], F32, tag="row")
                pend = None
                for m in range(JC):
                    u_ps = pbig.tile([128, QT], F32, tag="big")
                    for dc in range(DC):
                        nc.tensor.matmul(
                            u_ps, lhsT=wup_sb[:, dc, m * 128:(m + 1) * 128],
                            rhs=comb_t[:, dc, :],
                            start=(dc == 0), stop=(dc == DC - 1))
                    if pend is not None:
                        nc.tensor.matmul(ssu_ps, lhsT=ones_col, rhs=pend,
                                         start=(m == 1), stop=False)
                    nc.vector.tensor_copy(up_t[:, m, :], u_ps)
                    u2 = pu2.tile([128, QT], F32R, tag="u2")
                    nc.scalar.activation(u2, u_ps, AF.Square)
                    pend = u2
                nc.tensor.matmul(ssu_ps, lhsT=ones_col, rhs=pend,
                                 start=False, stop=True)
                sdu = prows.tile([1, QT], F32, tag="rows")
                nc.scalar.activation(sdu, ssu_ps, AF.Sqrt, bias=eps_t[:1],
                                     scale=1.0 / D_UP)
                r2f = prows.tile([1, QT], F32, tag="rows")
                nc.vector.reciprocal(r2f, sdu)
                r2 = prows.tile([1, QT], F32R, tag="rowsr")
                nc.vector.tensor_copy(r2, r2f)
                i2_b = pbc.tile([128, QT], F32, tag="bc")

                # ---- mix: out[mc] = i2_b * (Wmix_up @ up) + (Wmix_x @ x)
                for mc in range(MC):
                    wm_t = pwm.tile([128, KC, 128], F32R, tag="wm")
                    nc.sync.dma_start(wm_t[:], wmix_in.ap()[mc])
                    a_ps = pbig.tile([128, QT], F32, tag="big")
                    for kc in range(JC):
                        nc.tensor.matmul(a_ps, lhsT=wm_t[:, kc, :],
                                         rhs=up_t[:, kc, :],
                                         start=(kc == 0), stop=(kc == JC - 1))
                    b_ps = pbig.tile([128, QT], F32, tag="big")
                    for kc in range(MC):
                        nc.tensor.matmul(b_ps, lhsT=wm_t[:, JC + kc, :],
                                         rhs=x_t[:, kc, :],
                                         start=(kc == 0), stop=(kc == MC - 1))
                    if mc == 0:
                        i2_ps = pbig.tile([128, QT], F32, tag="big")
                        nc.tensor.matmul(i2_ps, lhsT=ones_row, rhs=r2,
                                         start=True, stop=True)
                        nc.vector.tensor_copy(i2_b, i2_ps)
                    o_sb = po.tile([128, QT], F32, tag="o")
                    nc.vector.tensor_tensor(o_sb, a_ps, i2_b, MUL)
                    nc.vector.tensor_tensor(o_sb, o_sb, b_ps, ADD)
                    nc.sync.dma_start(out_d.ap()[mc][:, qs], o_sb[:])

    nc.compile()
    return nc


def _get_program(W):
    if W not in _PROGRAM_CACHE:
        _PROGRAM_CACHE[W] = _build_program(W)
    return _PROGRAM_CACHE[W]


def kernel(**inputs) -> np.ndarray:
    global LAST_RESULTS
    inp = np.asarray(inputs["input"], np.float32)
    fw = np.asarray(inputs["fw"]).astype(np.int64)
    seq_sort = np.asarray(inputs["seq_sort"]).astype(np.int64)
    keep_cols = np.asarray(inputs["keep_cols"]).astype(np.int64)
    emb_alloc = np.asarray(inputs["emb_alloc"]).astype(np.int64)
    starts = np.asarray(inputs["starts"]).astype(np.int64)
    ends = np.asarray(inputs["ends"]).astype(np.int64)
    bb = int(np.asarray(inputs["bb"]))
    w_k = np.asarray(inputs["w_k_weight"], np.float32)
    w_v = np.asarray(inputs["w_v_weight"], np.float32)
    w_up = np.asarray(inputs["w_up_weight"], np.float32)
    w_mix = np.asarray(inputs["w_mix_weight"], np.float32)
    w_in = np.asarray(inputs["norm_in_weight"], np.float32)
    w_out = np.asarray(inputs["norm_out_weight"], np.float32)

    x = inp.reshape(BT, H)
    nb = BT // bb
    st = starts.reshape(nb, bb).min(axis=1)
    en = ends.reshape(nb, bb).max(axis=1)

    # sort block-rows j by label (stable); row s of sorted space = block-row
    # order[s] = query fw[order[s]]
    order = np.argsort(seq_sort, kind="stable")
    perm = fw[order]                         # original flat query per sorted row
    lab_q = seq_sort[order]                  # label per sorted row
    blk_q = order // bb
    st_q = st[blk_q]
    en_q = en[blk_q]
    x_sorted = x[perm]                       # [BT, H]

    # kv side: keep + label-sort; fold norm_in into K
    la = emb_alloc[keep_cols]                # [M]
    M = la.shape[0]
    kv_order = np.argsort(la, kind="stable")
    la_s = la[kv_order]
    kvpos = kv_order                         # kept-position of sorted kv row
    Bm = (w_k[keep_cols] * w_in[None, :])[kv_order]   # [M, H]
    Cm = w_v[keep_cols][kv_order]            # [M, D_EMB]

    counts = np.bincount(la_s, minlength=64)
    gstart = np.concatenate([[0], np.cumsum(counts)])  # [65]

    # per-tile windows over sorted kv
    NT = BT // QT                            # 32 global q-tiles
    win = np.empty(NT, np.int64)
    need = 0
    for g in range(NT):
        l0 = lab_q[g * QT]
        l1 = lab_q[(g + 1) * QT - 1]
        win[g] = gstart[l0]
        need = max(need, gstart[l1 + 1] - gstart[l0])
    W = max(256, int(-(-need // 128) * 128))

    # padded kv arrays so windows never go OOB
    Mp = M + W
    Bm_p = np.zeros((Mp, H), np.float32); Bm_p[:M] = Bm
    Cm_p = np.zeros((Mp, D_EMB), np.float32); Cm_p[:M] = Cm
    la_p = np.full(Mp, -1, np.int64); la_p[:M] = la_s
    kvpos_p = np.full(Mp, -1, np.int64); kvpos_p[:M] = kvpos

    # mask bias per (sorted row, window col)
    kvi = win[:, None] + np.arange(W)[None, :]           # [NT, W]
    la_w = la_p[kvi]                                     # [NT, W]
    kp_w = kvpos_p[kvi]
    lab_t = lab_q.reshape(NT, QT)
    st_t = st_q.reshape(NT, QT)
    en_t = en_q.reshape(NT, QT)
    valid = ((la_w[:, None, :] == lab_t[:, :, None])
             & (kp_w[:, None, :] >= st_t[:, :, None])
             & (kp_w[:, None, :] < en_t[:, :, None]))    # [NT, QT, W]
    bias = np.where(valid, np.float32(0), np.float32(-1e30))

    KT_full = np.ascontiguousarray(Bm_p.T)               # [H, Mp]

    wm = w_mix.copy()
    wm[:, :D_UP] *= w_out[None, :]
    WmixT = np.ascontiguousarray(wm.T)                   # [3072, H]
    wmix_host = np.ascontiguousarray(
        WmixT.reshape(KC, 128, MC, 128).transpose(2, 1, 0, 3))  # [MC,128,KC,128]
    WupT = np.ascontiguousarray(w_up.T)                  # [D_EMB, D_UP]
    wup_host = np.ascontiguousarray(
        WupT.reshape(DC, 128, D_UP).transpose(1, 0, 2))  # [128, DC, D_UP]

    n_kvc = W // 128
    in_maps = []
    for c in range(NC):
        rows = slice(c * NQ, (c + 1) * NQ)
        x_c = np.ascontiguousarray(
            x_sorted[rows].T.reshape(HC, 128, NQ).transpose(1, 0, 2))  # [128,HC,NQ]
        kt_c = np.empty((NQT, 128, HC, W), np.float32)
        v_c = np.empty((NQT, 128, n_kvc, D_EMB), np.float32)
        b_c = np.empty((NQT, 128, n_kvc, QT), np.float32)
        for qt in range(NQT):
            g = c * NQT + qt
            w0 = win[g]
            kt_c[qt] = KT_full[:, w0:w0 + W].reshape(HC, 128, W).transpose(1, 0, 2)
            v_c[qt] = Cm_p[w0:w0 + W].reshape(n_kvc, 128, D_EMB).transpose(1, 0, 2)
            b_c[qt] = bias[g].T.reshape(n_kvc, 128, QT).transpose(1, 0, 2)
        in_maps.append({
            "x_in": x_c, "kt_in": kt_c, "v_in": v_c, "b_in": b_c,
            "wup_in": wup_host, "wmix_in": wmix_host,
        })

    nc = _get_program(W)
    import time as _time
    global LAST_EXEC_S
    _t0 = _time.time()
    LAST_RESULTS = bass_utils.run_bass_kernel_spmd(nc, in_maps,
                                                   core_ids=list(range(NC)))
    LAST_EXEC_S = _time.time() - _t0
    out_sorted = np.concatenate(
        [r["out_d"].transpose(2, 0, 1).reshape(NQ, H) for r in LAST_RESULTS.results],
        axis=0)                                          # [BT, H]
    final = np.empty((BT, H), np.float32)
    final[perm] = out_sorted
    return final.reshape(B, T, H)


# revision 9
# speedup vs baseline: 54605.2471x; 54605.2471x over previous
"""Trainium2 Bass kernel for nn_L3_31799937859925 (sparse_attention).

Strategy:
- Each query row (label = seq_sort[j] in [0,64)) attends only to kv rows with
  emb_alloc == label, so we sort queries by label on the host and give each of
  the 8 cores a contiguous 2048-query slice (pure data parallel, no
  collectives). kv rows are label-sorted too, so each 512-query tile only needs
  a small contiguous kv window (W columns) + an additive -1e30 mask bias.
- On device everything is feature-major ([feature, query]) so no transposes are
  needed: scoresT = K'T @ x, softmax sums / rms stats via ones-column matmuls
  on the PE, per-query scalars broadcast across partitions via K=1 matmuls.
- norm_in_weight is folded into w_k, norm_out_weight into w_mix (host side).
- All heavy matmuls run in float32r (relaxed fp32, full PE rate, ~1.5e-4 rel).
"""
import numpy as np

import concourse.bass as bass
import concourse.tile as tile
from concourse import bacc, mybir
import concourse.bass_utils as bass_utils

F32 = mybir.dt.float32
F32R = mybir.dt.float32r
AF = mybir.ActivationFunctionType
MUL = mybir.AluOpType.mult
ADD = mybir.AluOpType.add

H, N_EMB, D_EMB, D_UP = 1024, 8192, 512, 2048
B, T = 4, 4096
BT = B * T                  # 16384
NC = 8                      # cores
NQ = BT // NC               # 2048 queries per core
QT = 512                    # queries per q-tile
NQT = NQ // QT              # 4 q-tiles per core
HC = H // 128               # 8
DC = D_EMB // 128           # 4
JC = D_UP // 128            # 16
KC = (D_UP + H) // 128      # 24 contraction chunks for mix
MC = H // 128               # 8 output chunks

LAST_RESULTS = None         # BassKernelResults of the most recent run (for test.py)
LAST_EXEC_S = None
_PROGRAM_CACHE = {}


def _build_program(W):
    """Build the SPMD single-core program. W = kv window width (mult of 128)."""
    n_kvc = W // 128
    nc = bacc.Bacc("TRN2", target_bir_lowering=False, debug=False,
                   enable_asserts=False)

    x_in = nc.dram_tensor("x_in", [128, HC, NQ], F32R, kind="ExternalInput")
    kt_in = nc.dram_tensor("kt_in", [NQT, 128, HC, W], F32R, kind="ExternalInput")
    v_in = nc.dram_tensor("v_in", [NQT, 128, n_kvc, D_EMB], F32R, kind="ExternalInput")
    b_in = nc.dram_tensor("b_in", [NQT, 128, n_kvc, QT], F32, kind="ExternalInput")
    wup_in = nc.dram_tensor("wup_in", [128, DC, D_UP], F32R, kind="ExternalInput")
    wmix_in = nc.dram_tensor("wmix_in", [MC, 128, KC, 128], F32R, kind="ExternalInput")
    out_d = nc.dram_tensor("out_d", [MC, 128, NQ], F32, kind="ExternalOutput")

    from contextlib import ExitStack
    with tile.TileContext(nc) as tc, ExitStack() as ctx:
        ec = ctx.enter_context
        cst = ec(tc.tile_pool(name="cst", bufs=1))
        pwup = ec(tc.tile_pool(name="wup", bufs=1))
        px = ec(tc.tile_pool(name="px", bufs=2))
        pkt = ec(tc.tile_pool(name="pkt", bufs=1))
        pv = ec(tc.tile_pool(name="pv", bufs=1))
        pb = ec(tc.tile_pool(name="pb", bufs=1))
        pwm = ec(tc.tile_pool(name="pwm", bufs=3))
        px2 = ec(tc.tile_pool(name="px2", bufs=2))
        ppu = ec(tc.tile_pool(name="ppu", bufs=1))
        pt = ec(tc.tile_pool(name="pt", bufs=3))
        pcomb = ec(tc.tile_pool(name="pcomb", bufs=1))
        pup = ec(tc.tile_pool(name="pup", bufs=1))
        pu2 = ec(tc.tile_pool(name="pu2", bufs=4))
        pbc = ec(tc.tile_pool(name="pbc", bufs=4))
        prows = ec(tc.tile_pool(name="prows", bufs=3))
        po = ec(tc.tile_pool(name="po", bufs=2))
        pbig = ec(tc.tile_pool(name="pbig", bufs=6, space="PSUM"))
        prow = ec(tc.tile_pool(name="prow", bufs=2, space="PSUM"))

        if True:
            ones_f = cst.tile([128, 1], F32)
            nc.vector.memset(ones_f, 1.0)
            ones_col = cst.tile([128, 1], F32R)
            nc.vector.tensor_copy(ones_col, ones_f)
            ones_rf = cst.tile([1, 128], F32)
            nc.vector.memset(ones_rf, 1.0)
            ones_row = cst.tile([1, 128], F32R)
            nc.vector.tensor_copy(ones_row, ones_rf)
            eps_t = cst.tile([128, 1], F32)
            nc.vector.memset(eps_t, 1e-6)

            wup_sb = pwup.tile([128, DC, D_UP], F32R)
            nc.sync.dma_start(wup_sb[:], wup_in.ap())

            for qt in range(NQT):
                qs = slice(qt * QT, (qt + 1) * QT)
                x_t = px.tile([128, HC, QT], F32R, tag="x")
                nc.sync.dma_start(x_t[:], x_in.ap()[:, :, qs])
                kt_t = pkt.tile([128, HC, W], F32R, tag="kt")
                nc.sync.dma_start(kt_t[:], kt_in.ap()[qt])
                v_t = pv.tile([128, n_kvc, D_EMB], F32R, tag="v")
                nc.sync.dma_start(v_t[:], v_in.ap()[qt])
                b_t = pb.tile([128, n_kvc, QT], F32, tag="b")
                nc.sync.dma_start(b_t[:], b_in.ap()[qt])

                # ---- rms_in stats: inv_rms per query as broadcast [128, QT]
                ss_ps = prow.tile([1, QT], F32, tag="row")
                for hc in range(HC):
                    x2 = px2.tile([128, QT], F32R, tag="x2")
                    nc.scalar.activation(x2, x_t[:, hc, :].bitcast(F32), AF.Square)
                    nc.tensor.matmul(ss_ps, lhsT=ones_col, rhs=x2,
                                     start=(hc == 0), stop=(hc == HC - 1))
                sd = prows.tile([1, QT], F32, tag="rows")
                nc.scalar.activation(sd, ss_ps, AF.Sqrt, bias=eps_t[:1],
                                     scale=1.0 / H)
                crf = prows.tile([1, QT], F32, tag="rows")
                nc.vector.reciprocal(crf, sd)
                cr = prows.tile([1, QT], F32R, tag="rowsr")
                nc.vector.tensor_copy(cr, crf)
                c_b = pbc.tile([128, QT], F32, tag="bc")

                # ---- scoresT [kv, q] per kv chunk; t = s*c + bias; pu = exp(t)
                pu_t = ppu.tile([128, n_kvc, QT], F32R, tag="pu")
                for kvc in range(n_kvc):
                    s_ps = pbig.tile([128, QT], F32, tag="big")
                    for hc in range(HC):
                        nc.tensor.matmul(
                            s_ps, lhsT=kt_t[:, hc, kvc * 128:(kvc + 1) * 128],
                            rhs=x_t[:, hc, :],
                            start=(hc == 0), stop=(hc == HC - 1))
                    if kvc == 0:
                        # emit bcast here so PE doesn't stall on the recip chain
                        cb_ps = pbig.tile([128, QT], F32, tag="big")
                        nc.tensor.matmul(cb_ps, lhsT=ones_row, rhs=cr,
                                         start=True, stop=True)
                        nc.vector.tensor_copy(c_b, cb_ps)
                    t_sb = pt.tile([128, QT], F32, tag="t")
                    nc.vector.tensor_tensor(t_sb, s_ps, c_b, MUL)
                    nc.vector.tensor_tensor(t_sb, t_sb, b_t[:, kvc, :], ADD)
                    nc.scalar.activation(pu_t[:, kvc, :], t_sb, AF.Exp)

                # ---- z = sum_kv pu ; z_b = 1/z broadcast
                z_ps = prow.tile([1, QT], F32, tag="row")
                for kvc in range(n_kvc):
                    nc.tensor.matmul(z_ps, lhsT=ones_col, rhs=pu_t[:, kvc, :],
                                     start=(kvc == 0), stop=(kvc == n_kvc - 1))
                zrf = prows.tile([1, QT], F32, tag="rows")
                nc.vector.reciprocal(zrf, z_ps)
                zr = prows.tile([1, QT], F32R, tag="rowsr")
                nc.vector.tensor_copy(zr, zrf)
                z_b = pbc.tile([128, QT], F32, tag="bc")

                # ---- combT [d, q] = V^T pu, normalized by z
                comb_t = pcomb.tile([128, DC, QT], F32R, tag="comb")
                for dc in range(DC):
                    c_ps = pbig.tile([128, QT], F32, tag="big")
                    for kvc in range(n_kvc):
                        nc.tensor.matmul(
                            c_ps, lhsT=v_t[:, kvc, dc * 128:(dc + 1) * 128],
                            rhs=pu_t[:, kvc, :],
                            start=(kvc == 0), stop=(kvc == n_kvc - 1))
                    if dc == 0:
                        zb_ps = pbig.tile([128, QT], F32, tag="big")
                        nc.tensor.matmul(zb_ps, lhsT=ones_row, rhs=zr,
                                         start=True, stop=True)
                        nc.vector.tensor_copy(z_b, zb_ps)
                    nc.vector.tensor_tensor(comb_t[:, dc, :], c_ps, z_b, MUL)

                # ---- upT [j, q] (raw, pre-norm) + sum of squares
                up_t = pup.tile([128, JC, QT], F32R, tag="up")
                ssu_ps = prow.tile([1, QT], F32, tag="row")
                pend = None
                for m in range(JC):
                    u_ps = pbig.tile([128, QT], F32, tag="big")
                    for dc in range(DC):
                        nc.tensor.matmul(
                            u_ps, lhsT=wup_sb[:, dc, m * 128:(m + 1) * 128],
                            rhs=comb_t[:, dc, :],
                            start=(dc == 0), stop=(dc == DC - 1))
                    if pend is not None:
                        nc.tensor.matmul(ssu_ps, lhsT=ones_col, rhs=pend,
                                         start=(m == 1), stop=False)
                    nc.vector.tensor_copy(up_t[:, m, :], u_ps)
                    u2 = pu2.tile([128, QT], F32R, tag="u2")
                    nc.scalar.activation(u2, u_ps, AF.Square)
                    pend = u2
                nc.tensor.matmul(ssu_ps, lhsT=ones_col, rhs=pend,
                                 start=False, stop=True)
                sdu = prows.tile([1, QT], F32, tag="rows")
                nc.scalar.activation(sdu, ssu_ps, AF.Sqrt, bias=eps_t[:1],
                                     scale=1.0 / D_UP)
                r2f = prows.tile([1, QT], F32, tag="rows")
                nc.vector.reciprocal(r2f, sdu)
                r2 = prows.tile([1, QT], F32R, tag="rowsr")
                nc.vector.tensor_copy(r2, r2f)
                i2_b = pbc.tile([128, QT], F32, tag="bc")

                # ---- mix: out[mc] = i2_b * (Wmix_up @ up) + (Wmix_x @ x)
                for mc in range(MC):
                    wm_t = pwm.tile([128, KC, 128], F32R, tag="wm")
                    nc.sync.dma_start(wm_t[:], wmix_in.ap()[mc])
                    a_ps = pbig.tile([128, QT], F32, tag="big")
                    for kc in range(JC):
                        nc.tensor.matmul(a_ps, lhsT=wm_t[:, kc, :],
                                         rhs=up_t[:, kc, :],
                                         start=(kc == 0), stop=(kc == JC - 1))
                    b_ps = pbig.tile([128, QT], F32, tag="big")
                    for kc in range(MC):
                        nc.tensor.matmul(b_ps, lhsT=wm_t[:, JC + kc, :],
                                         rhs=x_t[:, kc, :],
                                         start=(kc == 0), stop=(kc == MC - 1))
                    if mc == 0:
                        i2_ps = pbig.tile([128, QT], F32, tag="big")
                        nc.tensor.matmul(i2_ps, lhsT=ones_row, rhs=r2,
                                         start=True, stop=True)
                        nc.vector.tensor_copy(i2_b, i2_ps)
                    o_sb = po.tile([128, QT], F32, tag="o")
                    nc.vector.tensor_tensor(o_sb, a_ps, i2_b, MUL)
                    nc.vector.tensor_tensor(o_sb, o_sb, b_ps, ADD)
                    nc.sync.dma_start(out_d.ap()[mc][:, qs], o_sb[:])

    nc.compile()
    return nc


def _get_program(W):
    if W not in _PROGRAM_CACHE:
        _PROGRAM_CACHE[W] = _build_program(W)
    return _PROGRAM_CACHE[W]


def kernel(**inputs) -> np.ndarray:
    global LAST_RESULTS
    inp = np.asarray(inputs["input"], np.float32)
    fw = np.asarray(inputs["fw"]).astype(np.int64)
    seq_sort = np.asarray(inputs["seq_sort"]).astype(np.int64)
    keep_cols = np.asarray(inputs["keep_cols"]).astype(np.int64)
    emb_alloc = np.asarray(inputs["emb_alloc"]).astype(np.int64)
    starts = np.asarray(inputs["starts"]).astype(np.int64)
    ends = np.asarray(inputs["ends"]).astype(np.int64)
    bb = int(np.asarray(inputs["bb"]))
    w_k = np.asarray(inputs["w_k_weight"], np.float32)
    w_v = np.asarray(inputs["w_v_weight"], np.float32)
    w_up = np.asarray(inputs["w_up_weight"], np.float32)
    w_mix = np.asarray(inputs["w_mix_weight"], np.float32)
    w_in = np.asarray(inputs["norm_in_weight"], np.float32)
    w_out = np.asarray(inputs["norm_out_weight"], np.float32)

    x = inp.reshape(BT, H)
    nb = BT // bb
    st = starts.reshape(nb, bb).min(axis=1)
    en = ends.reshape(nb, bb).max(axis=1)

    # sort block-rows j by label (stable); row s of sorted space = block-row
    # order[s] = query fw[order[s]]
    order = np.argsort(seq_sort, kind="stable")
    perm = fw[order]                         # original flat query per sorted row
    lab_q = seq_sort[order]                  # label per sorted row
    blk_q = order // bb
    st_q = st[blk_q]
    en_q = en[blk_q]
    x_sorted = x[perm]                       # [BT, H]

    # kv side: keep + label-sort; fold norm_in into K
    la = emb_alloc[keep_cols]                # [M]
    M = la.shape[0]
    kv_order = np.argsort(la, kind="stable")
    la_s = la[kv_order]
    kvpos = kv_order                         # kept-position of sorted kv row
    Bm = (w_k[keep_cols] * w_in[None, :])[kv_order]   # [M, H]
    Cm = w_v[keep_cols][kv_order]            # [M, D_EMB]

    counts = np.bincount(la_s, minlength=64)
    gstart = np.concatenate([[0], np.cumsum(counts)])  # [65]

    # per-tile windows over sorted kv
    NT = BT // QT                            # 32 global q-tiles
    win = np.empty(NT, np.int64)
    need = 0
    for g in range(NT):
        l0 = lab_q[g * QT]
        l1 = lab_q[(g + 1) * QT - 1]
        win[g] = gstart[l0]
        need = max(need, gstart[l1 + 1] - gstart[l0])
    W = max(256, int(-(-need // 128) * 128))

    # padded kv arrays so windows never go OOB
    Mp = M + W
    Bm_p = np.zeros((Mp, H), np.float32); Bm_p[:M] = Bm
    Cm_p = np.zeros((Mp, D_EMB), np.float32); Cm_p[:M] = Cm
    la_p = np.full(Mp, -1, np.int64); la_p[:M] = la_s
    kvpos_p = np.full(Mp, -1, np.int64); kvpos_p[:M] = kvpos

    # mask bias per (sorted row, window col)
    kvi = win[:, None] + np.arange(W)[None, :]           # [NT, W]
    la_w = la_p[kvi]                                     # [NT, W]
    kp_w = kvpos_p[kvi]
    lab_t = lab_q.reshape(NT, QT)
    st_t = st_q.reshape(NT, QT)
    en_t = en_q.reshape(NT, QT)
    valid = ((la_w[:, None, :] == lab_t[:, :, None])
             & (kp_w[:, None, :] >= st_t[:, :, None])
             & (kp_w[:, None, :] < en_t[:, :, None]))    # [NT, QT, W]
    bias = np.where(valid, np.float32(0), np.float32(-1e30))

    KT_full = np.ascontiguousarray(Bm_p.T)               # [H, Mp]

    wm = w_mix.copy()
    wm[:, :D_UP] *= w_out[None, :]
    WmixT = np.ascontiguousarray(wm.T)                   # [3072, H]
    wmix_host = np.ascontiguousarray(
        WmixT.reshape(KC, 128, MC, 128).transpose(2, 1, 0, 3))  # [MC,128,KC,128]
    WupT = np.ascontiguousarray(w_up.T)                  # [D_EMB, D_UP]
    wup_host = np.ascontiguousarray(
        WupT.reshape(DC, 128, D_UP).transpose(1, 0, 2))  # [128, DC, D_UP]

    n_kvc = W // 128
    in_maps = []
    for c in range(NC):
        rows = slice(c * NQ, (c + 1) * NQ)
        x_c = np.ascontiguousarray(
            x_sorted[rows].T.reshape(HC, 128, NQ).transpose(1, 0, 2))  # [128,HC,NQ]
        kt_c = np.empty((NQT, 128, HC, W), np.float32)
        v_c = np.empty((NQT, 128, n_kvc, D_EMB), np.float32)
        b_c = np.empty((NQT, 128, n_kvc, QT), np.float32)
        for qt in range(NQT):
            g = c * NQT + qt
            w0 = win[g]
            kt_c[qt] = KT_full[:, w0:w0 + W].reshape(HC, 128, W).transpose(1, 0, 2)
            v_c[qt] = Cm_p[w0:w0 + W].reshape(n_kvc, 128, D_EMB).transpose(1, 0, 2)
            b_c[qt] = bias[g].T.reshape(n_kvc, 128, QT).transpose(1, 0, 2)
        in_maps.append({
            "x_in": x_c, "kt_in": kt_c, "v_in": v_c, "b_in": b_c,
            "wup_in": wup_host, "wmix_in": wmix_host,
        })

    nc = _get_program(W)
    import time as _time
    global LAST_EXEC_S
    _t0 = _time.time()
    LAST_RESULTS = bass_utils.run_bass_kernel_spmd(nc, in_maps,
                                                   core_ids=list(range(NC)))
    LAST_EXEC_S = _time.time() - _t0
    out_sorted = np.concatenate(
        [r["out_d"].transpose(2, 0, 1).reshape(NQ, H) for r in LAST_RESULTS.results],
        axis=0)                                          # [BT, H]
    final = np.empty((BT, H), np.float32)
    final[perm] = out_sorted
    return final.reshape(B, T, H)
